# revision 31
# baseline (speedup 1.0000x reference)
"""Trainium2 Bass kernel for causal GQA attention (B=2, S=2048, D=2048,
H=32, KVH=8, hd=64) with RoPE and output projection, running SPMD on 8
NeuronCores.

Sharding: tensor-parallel over heads (4-way) x data-parallel over batch
(2-way).  Core c (b = c//4, k = c%4) handles batch b, q heads 8k..8k+8
and kv heads 2k, 2k+1.

v2 design (vs the two-phase baseline):
  * bf16 everywhere on the matmul data path (x, weights, q/k/v, exp
    scores, collectives, wo) -- psum stays f32, output f32.
  * Head pairing ACROSS kv groups: pair m = (local head m, local head
    m+4), hf=0 on partitions 0:64 against kv group 0, hf=1 on 64:128
    against group 1.  The two score matmuls of a pair live on disjoint
    PE row groups and run concurrently; K needs no column duplication.
  * Single software-pipelined loop: projections+RoPE+V-transpose for
    s-tile t+1 and the wo matmuls for tile t-1 are chopped into small
    "filler" chunks and interleaved between the score groups of
    attention tile t, so the PE never idles while the scalar engine
    works through the exp activations (the true attention bottleneck).
  * AllGathers in bf16 trigger as each half-tile's heads finish; cct
    loads stay on the gpsimd queue but are double buffered so triggers
    never wait on loads.
"""

import numpy as np

DIM = 2048
S = 2048
B = 2
H = 32
KVH = 8
HD = 64
P = 128
QT = 512        # q tile (free dim of most matmuls)
NQT = S // QT   # 4
NKV = S // P    # 16 kv tiles of 128
DK = DIM // P   # 16 contraction tiles
ROPE_BASE = 10000.0
N_CORES = 8

_CACHE = {}


def _build():
    from collections import deque

    import concourse.bacc as bacc
    import concourse.tile as tile
    import concourse.mybir as mybir
    from concourse.masks import make_identity

    F32 = mybir.dt.float32
    BF = mybir.dt.bfloat16
    Exp = mybir.ActivationFunctionType.Exp

    nc = bacc.Bacc("TRN2", target_bir_lowering=False, debug=False,
                   num_devices=N_CORES)

    # host pre-tiles every matmul operand to [p, o, free] so each DMA
    # reads multi-KB contiguous lines per partition (the flat [dim, f]
    # layout costs 256B-1KB descriptors and starves the prologue)
    xT = nc.dram_tensor("xT", [NQT, P, DK, QT], BF,
                        kind="ExternalInput").ap()
    wqT = nc.dram_tensor("wqT", [4, P, DK, P], BF,
                         kind="ExternalInput").ap()
    wkT = nc.dram_tensor("wkT", [P, DK, P], BF, kind="ExternalInput").ap()
    wvT = nc.dram_tensor("wvT", [P, DK, P], BF, kind="ExternalInput").ap()
    woT = nc.dram_tensor("woT", [P, DK, 512], BF, kind="ExternalInput").ap()
    cosT = nc.dram_tensor("cosT", [P, S], F32, kind="ExternalInput").ap()
    sinT = nc.dram_tensor("sinT", [P, S], F32, kind="ExternalInput").ap()
    maskT = nc.dram_tensor("maskT", [P, 4, QT], BF, kind="ExternalInput").ap()
    out_t = nc.dram_tensor("out_t", [512, S], F32, kind="ExternalOutput").ap()

    with tile.TileContext(nc) as tc:
        with (
            tc.tile_pool(name="pers", bufs=1) as pers,
            tc.tile_pool(name="run", bufs=1) as run,
            tc.tile_pool(name="ps", bufs=1, space="PSUM") as ps,
            tc.tile_pool(name="dram", bufs=1, space="DRAM") as dram,
        ):
            # ---------------- persistent tiles ----------------
            q_fin = [pers.tile([P, S], BF, name=f"q_fin{m}") for m in range(4)]
            k_fin = pers.tile([P, S], BF, name="k_fin")
            vT_raw = pers.tile([P, S], BF, name="vT_raw")
            v1 = [pers.tile([P, NKV, P], BF, name=f"v1_{g}") for g in range(2)]
            msk = pers.tile([P, 4, QT], BF, name="msk")
            cos_sb = pers.tile([P, S], F32, name="cos_sb")
            sin_sb = pers.tile([P, S], F32, name="sin_sb")
            wq_sb = [pers.tile([P, DK, P], BF, name=f"wq_sb{m}")
                     for m in range(4)]
            wk_sb = pers.tile([P, DK, P], BF, name="wk_sb")
            wv_sb = pers.tile([P, DK, P], BF, name="wv_sb")
            wo_sb = pers.tile([P, DK, 512], BF, name="wo_sb")
            ident_f = pers.tile([P, P], F32, name="ident_f")
            ident = pers.tile([P, P], BF, name="ident")

            # tiles 0..2 gather per half (pairs 2hh, 2hh+1); tile 3
            # gathers per pair so the tail only waits on a tiny op.
            cc_in = [[dram.tile([256, QT], BF, name=f"cc_in{t}_{hh}")
                      for hh in range(2)] for t in range(NQT - 1)]
            cc_out = [[dram.tile([4 * 256, QT], BF, name=f"cc_out{t}_{hh}")
                       for hh in range(2)] for t in range(NQT - 1)]
            cc_in.append([dram.tile([P, QT], BF, name=f"cc_in3_{m}")
                          for m in range(4)])
            cc_out.append([dram.tile([4 * P, QT], BF, name=f"cc_out3_{m}")
                           for m in range(4)])

            # PSUM budget (8 banks): sc2 = 2 tiles x 2 banks (scores),
            # pv = 2 tiles x 1 bank (PV accumulators), wo = 2 tiles x
            # 1 bank (shared rotation: proj targets, wo d-slices, and
            # the V transposes).  The tail borrows one sc2 tile.
            def sc2(name):
                return ps.tile([P, 2, QT], F32, tag="sc2", bufs=2, name=name)

            def pvb(name):
                return ps.tile([P, QT], F32, tag="pv", bufs=2, name=name)

            def wob(name, shape=None, dtype=None):
                return ps.tile(shape or [P, QT], dtype or F32, tag="wo",
                               bufs=2, name=name)

            # ---------------- startup DMAs (spread queues) ----------------
            x_tiles = {}

            def x_load(st):
                xt = run.tile([P, DK, QT], BF, tag="xsb", bufs=2,
                              name=f"x{st}")
                for c in range(8):
                    nc.sync.dma_start(xt[:, 2 * c:2 * (c + 1), :],
                                      xT[st, :, 2 * c:2 * (c + 1), :])
                x_tiles[st] = xt

            x_load(0)
            for m in range(4):
                nc.scalar.dma_start(wq_sb[m][:], wqT[m])
            nc.scalar.dma_start(wk_sb[:], wkT[:])
            nc.scalar.dma_start(wv_sb[:], wvT[:])
            nc.scalar.dma_start(cos_sb[:], cosT[:])
            nc.scalar.dma_start(sin_sb[:], sinT[:])
            nc.gpsimd.dma_start(wo_sb[:], woT[:])
            nc.gpsimd.dma_start(msk[:], maskT[:])

            # warm the ACT exp table set (~2.7us) under the prologue
            warm = run.tile([1, 8], F32, tag="warm", bufs=1, name="warm")
            nc.vector.memset(warm[:], 0.0)
            nc.scalar.activation(warm[:], warm[:], Exp)

            make_identity(nc, ident_f[:])
            nc.vector.tensor_copy(ident[:], ident_f[:])

            ones3 = run.tile([P, NKV, HD], BF, tag="ones", bufs=1,
                             name="ones3")
            nc.vector.memset(ones3[:], 1.0)
            for g in range(2):
                nc.vector.tensor_copy(v1[g][:, :, 0:HD], ones3[:])

            # ---------------- projection machinery ----------------
            def rope_finalize(dst, pw, st):
                """psum -> RoPE -> dst[:, s-tile st] (dst bf16)."""
                ssl = slice(st * QT, (st + 1) * QT)
                raw = run.tile([P, QT], F32, tag="raw", bufs=3, name="raw")
                nc.vector.tensor_copy(raw[:], pw[:])
                rot = run.tile([P, QT], F32, tag="rot", bufs=2, name="rot")
                for hh in range(2):
                    base = hh * HD
                    nc.sync.dma_start(rot[base:base + 32, :],
                                      raw[base + 32:base + 64, :])
                    nc.sync.dma_start(rot[base + 32:base + 64, :],
                                      raw[base:base + 32, :])
                nc.vector.tensor_mul(rot[:], rot[:], sin_sb[:, ssl])
                nc.vector.tensor_mul(raw[:], raw[:], cos_sb[:, ssl])
                nc.vector.tensor_add(dst[:, ssl], raw[:], rot[:])

            def proj_chunks(st):
                """Filler chunks computing Q/K/V + RoPE for s-tile st."""
                xt = x_tiles[st]
                ssl = slice(st * QT, (st + 1) * QT)
                chunks = []
                state = {}

                def mk_mm(key, w, oc, fin):
                    def emit():
                        if oc == 0:
                            state[key] = wob(f"pj_{st}_{key}")
                        pw = state[key]
                        for oo in range(4):
                            o = 4 * oc + oo
                            nc.tensor.matmul(pw[:], w[:, o, :], xt[:, o, :],
                                             start=(o == 0), stop=(o == DK - 1))
                        if oc == 3:
                            fin(pw)
                    return emit

                def add_q(m):
                    for oc in range(4):
                        chunks.append(mk_mm(
                            f"q{m}", wq_sb[m], oc,
                            (lambda pw, m=m: rope_finalize(q_fin[m], pw, st))))

                # q0/k/v first so their RoPE + transpose chains finish
                # well before the next tile's first attention pair; the
                # transposes go after q1 so the vT_raw drain (DVE) has
                # time to complete under q1's matmuls.
                add_q(0)
                for oc in range(4):
                    chunks.append(mk_mm(
                        "k", wk_sb, oc,
                        lambda pw: rope_finalize(k_fin, pw, st)))
                for oc in range(4):
                    chunks.append(mk_mm(
                        "v", wv_sb, oc,
                        lambda pw: nc.vector.tensor_copy(vT_raw[:, ssl],
                                                         pw[:])))
                add_q(1)

                def mk_tr(j):
                    def emit():
                        pst = wob(f"pst_{st}_{j}", [P, P], BF)
                        nc.tensor.transpose(
                            pst[:], vT_raw[:, j * P:(j + 1) * P], ident[:])
                        for g in range(2):
                            nc.vector.tensor_copy(
                                v1[g][:, j, HD:P],
                                pst[:, g * HD:(g + 1) * HD])
                    return emit

                for jj in range(4):
                    chunks.append(mk_tr(4 * st + jj))
                add_q(2)
                add_q(3)
                return chunks

            # ---------------- wo machinery ----------------
            cct_tiles = {}

            def cct_load(t, u):
                """Load gathered unit u of tile t (half for t<3, pair
                for t=3) into sbuf.  wo_sb block index for gathered
                block (u, r): per-pair order 4m+r with m = 2hh+o2."""
                if t < NQT - 1:
                    cct = run.tile([P, 4, 2, QT], BF, tag="cct", bufs=2,
                                   name=f"cct{t}_{u}")
                    cc3 = cc_out[t][u][:].rearrange("(r o p) s -> p r o s",
                                                    p=P, o=2)
                else:
                    cct = run.tile([P, 4, QT], BF, tag="cct3", bufs=4,
                                   name=f"cct{t}_{u}")
                    cc3 = cc_out[t][u][:].rearrange("(r p) s -> p r s", p=P)
                nc.gpsimd.dma_start(cct[:], cc3[:])
                cct_tiles[(t, u)] = cct

            def wo_fin(t, d, pw):
                qsl = slice(t * QT, (t + 1) * QT)
                ot = run.tile([P, QT], F32, tag="ot", bufs=2, name="ot")
                nc.vector.tensor_copy(ot[:], pw[:])
                nc.sync.dma_start(out_t[d * P:(d + 1) * P, qsl], ot[:])

            def wo_chunks(t):
                """Filler chunks for the wo projection of q tile t<3,
                contracting the two gathered halves in sequence."""
                chunks = []
                state = {}

                def mk(d, hh, half, fin=False):
                    def emit():
                        if d not in state:
                            state[d] = wob(f"wo_{t}_d{d}")
                        pw = state[d]
                        cct = cct_tiles[(t, hh)]
                        for rr in range(2):
                            r = 2 * half + rr
                            for o2 in range(2):
                                nc.tensor.matmul(
                                    pw[:],
                                    wo_sb[:, 4 * (2 * hh + o2) + r,
                                          d * P:(d + 1) * P],
                                    cct[:, r, o2, :],
                                    start=(hh == 0 and r == 0 and o2 == 0),
                                    stop=(hh == 1 and r == 3 and o2 == 1))
                        if fin:
                            wo_fin(t, d, pw)
                    return emit

                early = [mk(0, 0, 0), mk(0, 0, 1), mk(1, 0, 0), mk(1, 0, 1)]
                late = [mk(0, 1, 0), mk(0, 1, 1, fin=True),
                        mk(1, 1, 0), mk(1, 1, 1, fin=True),
                        mk(2, 0, 0), mk(2, 0, 1), mk(3, 0, 0), mk(3, 0, 1),
                        mk(2, 1, 0), mk(2, 1, 1, fin=True),
                        mk(3, 1, 0), mk(3, 1, 1, fin=True)]
                return early, late

            def wo_tail_chunks(t):
                """wo for the last tile from the per-pair gathers."""
                chunks = []
                state = {}

                def mk(d, m, fin=False):
                    def emit():
                        if d not in state:
                            if d >= 2:
                                if "d23" not in state:
                                    state["d23"] = sc2(f"wo_{t}_d23")
                                state[d] = state["d23"][:, d - 2, :]
                            else:
                                state[d] = wob(f"wo_{t}_d{d}")
                        pw = state[d]
                        cct = cct_tiles[(t, m)]
                        for r in range(4):
                            nc.tensor.matmul(
                                pw[:],
                                wo_sb[:, 4 * m + r, d * P:(d + 1) * P],
                                cct[:, r, :],
                                start=(m == 0 and r == 0),
                                stop=(m == 3 and r == 3))
                        if fin:
                            wo_fin(t, d, pw)
                    return emit

                for m in range(3):
                    chunks += [mk(0, m), mk(1, m)]
                for m in range(3):
                    chunks += [mk(2, m), mk(3, m)]
                chunks += [mk(0, 3, fin=True), mk(1, 3, fin=True),
                           mk(2, 3, fin=True), mk(3, 3, fin=True)]
                return chunks

            # ---------------- attention ----------------
            def trig_ag(t, m):
                nc.gpsimd.collective_compute(
                    "AllGather",
                    mybir.AluOpType.bypass,
                    replica_groups=[[0, 1, 2, 3], [4, 5, 6, 7]],
                    ins=[cc_in[t][m][:].opt()],
                    outs=[cc_out[t][m][:].opt()],
                )

            prs = [slice(0, HD), slice(HD, P)]

            def attn_pair(t, m, pop):
                """Heads (m, m+4): hf=0 on partitions 0:64 vs kv group 0,
                hf=1 on 64:128 vs group 1 -- disjoint PE row groups, so
                each score matmul pair runs concurrently."""
                ngrp = 2 * (t + 1)
                qsl = slice(t * QT, (t + 1) * QT)
                pspv = [pvb(f"pv_{t}_{m}_{hf}") for hf in range(2)]
                e_pair = []
                for g2 in range(ngrp):
                    pss = [sc2(f"ss_{t}_{m}_{g2}_{hf}") for hf in range(2)]
                    for i in range(2):
                        j = 2 * g2 + i
                        for hf in range(2):
                            nc.tensor.matmul(
                                pss[hf][:, i, :],
                                k_fin[prs[hf], j * P:(j + 1) * P],
                                q_fin[m][prs[hf], qsl],
                                start=True, stop=True)
                    e2 = []
                    for hf in range(2):
                        e = run.tile([P, 2, QT], BF, tag="exp", bufs=6,
                                     name="e2")
                        nc.scalar.activation(e[:], pss[hf][:], Exp,
                                             scale=0.125)
                        cpair = g2 - 2 * t
                        if cpair >= 0:
                            nc.vector.tensor_mul(
                                e[:], e[:], msk[:, 2 * cpair:2 * cpair + 2, :])
                        e2.append(e)
                    e_pair.append(e2)
                    if g2 >= 1:
                        gp = g2 - 1
                        for i in range(2):
                            j = 2 * gp + i
                            for hf in range(2):
                                nc.tensor.matmul(
                                    pspv[hf][:], v1[hf][:, j, :],
                                    e_pair[gp][hf][:, i, :],
                                    start=(j == 0), stop=False)
                    pop()
                for i in range(2):
                    j = 2 * (ngrp - 1) + i
                    for hf in range(2):
                        nc.tensor.matmul(
                            pspv[hf][:], v1[hf][:, j, :],
                            e_pair[ngrp - 1][hf][:, i, :],
                            start=(j == 0), stop=(j == 4 * t + 3))
                # normalize: psum partitions 0:64 hold the row sums
                # (ones columns), 64:128 hold PV.
                for hf in range(2):
                    ocp = run.tile([P, QT], F32, tag="ocp", bufs=3,
                                   name="ocp")
                    nc.vector.tensor_copy(ocp[:], pspv[hf][:])
                    recip = run.tile([1, QT], F32, tag="recip", bufs=2,
                                     name="recip")
                    nc.vector.reciprocal_approx_fast(recip[:], ocp[0:1, :])
                    rb = dram.tile([1, QT], F32, tag="rb", bufs=2, name="rb")
                    nc.sync.dma_start(rb[:], recip[:])
                    bcast = run.tile([P, QT], F32, tag="bcast", bufs=2,
                                     name="bcast")
                    nc.sync.dma_start(bcast[HD:P, :],
                                      rb[:].to_broadcast((HD, QT)))
                    o_sb = run.tile([P, QT], BF, tag="osb", bufs=3,
                                    name="o_sb")
                    nc.vector.tensor_mul(o_sb[HD:P, :], ocp[HD:P, :],
                                         bcast[HD:P, :])
                    if t < NQT - 1:
                        row = (m % 2) * P + hf * HD
                        nc.sync.dma_start(
                            cc_in[t][m // 2][row:row + HD, :], o_sb[HD:P, :])
                    else:
                        nc.sync.dma_start(
                            cc_in[t][m][hf * HD:(hf + 1) * HD, :],
                            o_sb[HD:P, :])

            # ---------------- prologue: projections for s-tile 0 ------
            for emit in proj_chunks(0):
                emit()
            x_load(1)

            # ---------------- main pipelined loop ----------------
            for t in range(NQT):
                fillers = deque()
                wo_late = []
                if t < NQT - 1:
                    fillers.extend(proj_chunks(t + 1))
                if t >= 1:
                    early, wo_late = wo_chunks(t - 1)
                    fillers.extend(early)
                if t >= 1:
                    cct_load(t - 1, 0)
                if t + 2 < NQT:
                    x_load(t + 2)

                pops_total = 4 * 2 * (t + 1)
                pops_done = 0

                def pop():
                    nonlocal pops_done
                    pops_done += 1
                    left = pops_total - pops_done
                    if left <= 0:
                        while fillers:
                            fillers.popleft()()
                        return
                    k = -(-len(fillers) // (left + 1))
                    for _ in range(min(k, len(fillers))):
                        fillers.popleft()()

                # gpsimd queue: triggers interleave with the loads so a
                # slow AllGather's load-wait never delays a trigger
                # whose input is already on DRAM.
                last = t == NQT - 1
                for m in range(4):
                    attn_pair(t, m, pop)
                    if last:
                        trig_ag(t, m)
                        if m == 0 and t >= 1:
                            cct_load(t - 1, 1)
                            fillers.extend(wo_late)
                        if m == 2:
                            cct_load(t, 0)
                    else:
                        if m == 1:
                            trig_ag(t, 0)
                            if t >= 1:
                                cct_load(t - 1, 1)
                        if m == 2:
                            fillers.extend(wo_late)
                        if m == 3:
                            trig_ag(t, 1)
                while fillers:
                    fillers.popleft()()

            # ---------------- tail: wo for the last tile ----------------
            for m in range(1, 4):
                cct_load(NQT - 1, m)
            for emit in wo_tail_chunks(NQT - 1):
                emit()

    nc.compile()
    return nc


def _prep_inputs(x, position_ids, wq, wk, wv, wo):
    import ml_dtypes

    BF = ml_dtypes.bfloat16
    x = np.asarray(x, dtype=np.float32)
    pos = np.asarray(position_ids).reshape(-1).astype(np.int64)
    wqf = np.asarray(wq, dtype=np.float32)
    wkf = np.asarray(wk, dtype=np.float32)
    wvf = np.asarray(wv, dtype=np.float32)
    wof = np.asarray(wo, dtype=np.float32)

    inv = 1.0 / (ROPE_BASE ** (np.arange(0, HD, 2, dtype=np.float32) / HD))
    freqs = np.outer(pos.astype(np.float32), inv)  # [S, 32]
    pidx = np.arange(P) % 32
    sign = np.where((np.arange(P) % HD) < 32, -1.0, 1.0).astype(np.float32)
    cosT = np.ascontiguousarray(np.cos(freqs)[:, pidx].T)          # [P, S]
    sinT = np.ascontiguousarray(np.sin(freqs)[:, pidx].T * sign[:, None])

    pg = np.arange(P)[:, None, None]
    cg = np.arange(4)[None, :, None]
    fg = np.arange(QT)[None, None, :]
    maskT = ((fg - pg - 128 * cg) >= 0).astype(BF)

    def tile_po(wT, fw):
        """[dim, f] -> [p, o, f] with dim = o*128 + p, contiguous."""
        return np.ascontiguousarray(
            wT.reshape(DK, P, fw).transpose(1, 0, 2).astype(BF))

    # x pre-tiled to [st, p, o, s]
    xH = [np.ascontiguousarray(
        x[b].reshape(NQT, QT, DK, P).transpose(0, 3, 2, 1).astype(BF))
        for b in range(B)]

    in_maps = []
    for c in range(N_CORES):
        b, k = c // 4, c % 4
        # wq columns in head-paired order: block m = [head 8k+m | 8k+m+4]
        wq_t = np.empty((4, P, DK, P), dtype=BF)
        for m in range(4):
            qcols = []
            for hf in range(2):
                h = 8 * k + m + 4 * hf
                qcols.extend(range(h * HD, (h + 1) * HD))
            wq_t[m] = tile_po(wqf[qcols, :].T, P)
        wkT_loc = tile_po(wkf[2 * k * HD:(2 * k + 2) * HD, :].T, P)
        wvT_loc = tile_po(wvf[2 * k * HD:(2 * k + 2) * HD, :].T, P)
        # wo rows permuted to the per-pair gathered layout: block
        # o = 4m + r covers rows of head 8r + 4hf + m at hf*64+d.
        perm = []
        for m in range(4):
            for r in range(4):
                for hf in range(2):
                    h = 8 * r + 4 * hf + m
                    perm.extend(range(h * HD, (h + 1) * HD))
        woT_loc = tile_po(wof[512 * k:512 * (k + 1), perm].T, 512)
        in_maps.append({
            "xT": xH[b],
            "wqT": np.ascontiguousarray(wq_t),
            "wkT": wkT_loc,
            "wvT": wvT_loc,
            "woT": woT_loc,
            "cosT": cosT,
            "sinT": sinT,
            "maskT": maskT,
        })
    return in_maps


LAST_EXEC_NS = None


def kernel(x, position_ids, wq, wk, wv, wo, _trace=False):
    import time

    from concourse import bass_utils

    if "nc" not in _CACHE:
        _CACHE["nc"] = _build()
    nc = _CACHE["nc"]

    in_maps = _prep_inputs(x, position_ids, wq, wk, wv, wo)
    res = None
    for attempt in range(3):
        try:
            res = bass_utils.run_bass_kernel_spmd(
                nc, in_maps, core_ids=list(range(N_CORES)), trace=_trace)
            break
        except Exception:
            if attempt == 2:
                raise
            time.sleep(20 * (attempt + 1))

    global LAST_EXEC_NS
    LAST_EXEC_NS = res.exec_time_ns

    out = np.empty((B, S, DIM), dtype=np.float32)
    for c in range(N_CORES):
        b, k = c // 4, c % 4
        out[b, :, 512 * k:512 * (k + 1)] = res.results[c]["out_t"].T
    return out


# revision 39
# speedup vs baseline: 9562.0220x; 9562.0220x over previous
"""Trainium2 Bass kernel for causal GQA attention (B=2, S=2048, D=2048,
H=32, KVH=8, hd=64) with RoPE and output projection, running SPMD on 8
NeuronCores.

Sharding: tensor-parallel over heads (4-way) x data-parallel over batch
(2-way).  Core c (b = c//4, k = c%4) handles batch b, q heads 8k..8k+8
and kv heads 2k, 2k+1.

v2 design (vs the two-phase baseline):
  * bf16 everywhere on the matmul data path (x, weights, q/k/v, exp
    scores, collectives, wo) -- psum stays f32, output f32.
  * Head pairing ACROSS kv groups: pair m = (local head m, local head
    m+4), hf=0 on partitions 0:64 against kv group 0, hf=1 on 64:128
    against group 1.  The two score matmuls of a pair live on disjoint
    PE row groups and run concurrently; K needs no column duplication.
  * Single software-pipelined loop: projections+RoPE+V-transpose for
    s-tile t+1 and the wo matmuls for tile t-1 are chopped into small
    "filler" chunks and interleaved between the score groups of
    attention tile t, so the PE never idles while the scalar engine
    works through the exp activations (the true attention bottleneck).
  * AllGathers in bf16 trigger as each half-tile's heads finish; cct
    loads stay on the gpsimd queue but are double buffered so triggers
    never wait on loads.
"""

import numpy as np

DIM = 2048
S = 2048
B = 2
H = 32
KVH = 8
HD = 64
P = 128
QT = 512        # q tile (free dim of most matmuls)
NQT = S // QT   # 4
NKV = S // P    # 16 kv tiles of 128
DK = DIM // P   # 16 contraction tiles
ROPE_BASE = 10000.0
N_CORES = 8

_CACHE = {}


def _build():
    from collections import deque

    import concourse.bacc as bacc
    import concourse.tile as tile
    import concourse.mybir as mybir
    from concourse.masks import make_identity

    F32 = mybir.dt.float32
    BF = mybir.dt.bfloat16
    Exp = mybir.ActivationFunctionType.Exp

    nc = bacc.Bacc("TRN2", target_bir_lowering=False, debug=False,
                   num_devices=N_CORES)

    # host pre-tiles every matmul operand to [p, o, free] so each DMA
    # reads multi-KB contiguous lines per partition (the flat [dim, f]
    # layout costs 256B-1KB descriptors and starves the prologue)
    xT = nc.dram_tensor("xT", [NQT, P, DK, QT], BF,
                        kind="ExternalInput").ap()
    wqT = nc.dram_tensor("wqT", [P, 4 * DK, P], BF,
                         kind="ExternalInput").ap()
    wkvT = nc.dram_tensor("wkvT", [P, 2 * DK, P], BF,
                          kind="ExternalInput").ap()
    woT = nc.dram_tensor("woT", [P, DK, 512], BF, kind="ExternalInput").ap()
    csT = nc.dram_tensor("csT", [P, 2, S], F32, kind="ExternalInput").ap()
    maskT = nc.dram_tensor("maskT", [P, 4, QT], BF, kind="ExternalInput").ap()
    out_t = nc.dram_tensor("out_t", [512, S], F32, kind="ExternalOutput").ap()

    with tile.TileContext(nc) as tc:
        with (
            tc.tile_pool(name="pers", bufs=1) as pers,
            tc.tile_pool(name="run", bufs=1) as run,
            tc.tile_pool(name="ps", bufs=1, space="PSUM") as ps,
            tc.tile_pool(name="dram", bufs=1, space="DRAM") as dram,
        ):
            # ---------------- persistent tiles ----------------
            q_fin = [pers.tile([P, S], BF, name=f"q_fin{m}") for m in range(4)]
            k_fin = pers.tile([P, S], BF, name="k_fin")
            vT_raw = pers.tile([P, S], BF, name="vT_raw")
            v1 = [pers.tile([P, NKV, P], BF, name=f"v1_{g}") for g in range(2)]
            msk = pers.tile([P, 4, QT], BF, name="msk")
            cs_sb = pers.tile([P, 2, S], F32, name="cs_sb")
            wq_sb = pers.tile([P, 4 * DK, P], BF, name="wq_sb")
            wkv_sb = pers.tile([P, 2 * DK, P], BF, name="wkv_sb")
            wo_sb = pers.tile([P, DK, 512], BF, name="wo_sb")
            ident_f = pers.tile([P, P], F32, name="ident_f")
            ident = pers.tile([P, P], BF, name="ident")

            # tiles 0..2 gather per half (pairs 2hh, 2hh+1); tile 3
            # gathers per pair so the tail only waits on a tiny op.
            cc_in = [[dram.tile([256, QT], BF, name=f"cc_in{t}_{hh}")
                      for hh in range(2)] for t in range(NQT - 1)]
            cc_out = [[dram.tile([4 * 256, QT], BF, name=f"cc_out{t}_{hh}")
                       for hh in range(2)] for t in range(NQT - 1)]
            cc_in.append([dram.tile([P, QT], BF, name=f"cc_in3_{m}")
                          for m in range(4)])
            cc_out.append([dram.tile([4 * P, QT], BF, name=f"cc_out3_{m}")
                           for m in range(4)])

            # PSUM budget (8 banks): sc2 = 2 tiles x 2 banks (scores),
            # pv = 2 tiles x 1 bank (PV accumulators), wo = 2 tiles x
            # 1 bank (shared rotation: proj targets, wo d-slices, and
            # the V transposes).  The tail borrows one sc2 tile.
            def sc2(name):
                return ps.tile([P, 2, QT], F32, tag="sc2", bufs=2, name=name)

            def pvb(name):
                return ps.tile([P, QT], F32, tag="pv", bufs=2, name=name)

            def wob(name, shape=None, dtype=None):
                return ps.tile(shape or [P, QT], dtype or F32, tag="wo",
                               bufs=2, name=name)

            # ---------------- startup DMAs (spread queues) ----------------
            x_tiles = {}

            def x_load(st):
                xt = run.tile([P, DK, QT], BF, tag="xsb", bufs=2,
                              name=f"x{st}")
                nc.sync.dma_start(xt[:, 0:8, :], xT[st, :, 0:8, :])
                nc.sync.dma_start(xt[:, 8:16, :], xT[st, :, 8:16, :])
                x_tiles[st] = xt

            # one big-line DMA per operand group -- the rings are
            # descriptor-rate bound (~70ns/desc), so per-m / per-tensor
            # loads serialize the prologue for tens of us
            x_load(0)
            nc.scalar.dma_start(wq_sb[:], wqT[:])
            nc.scalar.dma_start(wkv_sb[:], wkvT[:])
            nc.scalar.dma_start(wo_sb[:], woT[:])
            nc.gpsimd.dma_start(cs_sb[:], csT[:])
            nc.gpsimd.dma_start(msk[:], maskT[:])

            # warm the ACT exp table set (~2.7us) under the prologue
            warm = run.tile([1, 8], F32, tag="warm", bufs=1, name="warm")
            nc.vector.memset(warm[:], 0.0)
            nc.scalar.activation(warm[:], warm[:], Exp)

            make_identity(nc, ident_f[:])
            nc.vector.tensor_copy(ident[:], ident_f[:])

            ones3 = run.tile([P, NKV, HD], BF, tag="ones", bufs=1,
                             name="ones3")
            nc.vector.memset(ones3[:], 1.0)
            for g in range(2):
                nc.vector.tensor_copy(v1[g][:, :, 0:HD], ones3[:])

            # ---------------- projection machinery ----------------
            def rope_finalize(dst, pw, st):
                """psum -> RoPE -> dst[:, s-tile st] (dst bf16)."""
                ssl = slice(st * QT, (st + 1) * QT)
                raw = run.tile([P, QT], F32, tag="raw", bufs=3, name="raw")
                nc.vector.tensor_copy(raw[:], pw[:])
                rot = run.tile([P, QT], F32, tag="rot", bufs=2, name="rot")
                for hh in range(2):
                    base = hh * HD
                    nc.sync.dma_start(rot[base:base + 32, :],
                                      raw[base + 32:base + 64, :])
                    nc.sync.dma_start(rot[base + 32:base + 64, :],
                                      raw[base:base + 32, :])
                nc.vector.tensor_mul(rot[:], rot[:], cs_sb[:, 1, ssl])
                nc.vector.tensor_mul(raw[:], raw[:], cs_sb[:, 0, ssl])
                nc.vector.tensor_add(dst[:, ssl], raw[:], rot[:])

            def proj_chunks(st):
                """Filler chunks computing Q/K/V + RoPE for s-tile st."""
                xt = x_tiles[st]
                ssl = slice(st * QT, (st + 1) * QT)
                chunks = []
                state = {}

                def mk_mm(key, wt, obase, oc, fin):
                    def emit():
                        if oc == 0:
                            state[key] = wob(f"pj_{st}_{key}")
                        pw = state[key]
                        for oo in range(4):
                            o = 4 * oc + oo
                            nc.tensor.matmul(pw[:], wt[:, obase + o, :],
                                             xt[:, o, :],
                                             start=(o == 0), stop=(o == DK - 1))
                        if oc == 3:
                            fin(pw)
                    return emit

                def add_q(m):
                    for oc in range(4):
                        chunks.append(mk_mm(
                            f"q{m}", wq_sb, m * DK, oc,
                            (lambda pw, m=m: rope_finalize(q_fin[m], pw, st))))

                # q0/k/v first so their RoPE + transpose chains finish
                # well before the next tile's first attention pair; the
                # transposes go after q1 so the vT_raw drain (DVE) has
                # time to complete under q1's matmuls.
                add_q(0)
                for oc in range(4):
                    chunks.append(mk_mm(
                        "k", wkv_sb, 0, oc,
                        lambda pw: rope_finalize(k_fin, pw, st)))
                for oc in range(4):
                    chunks.append(mk_mm(
                        "v", wkv_sb, DK, oc,
                        lambda pw: nc.vector.tensor_copy(vT_raw[:, ssl],
                                                         pw[:])))
                add_q(1)

                def mk_tr(j):
                    def emit():
                        pst = wob(f"pst_{st}_{j}", [P, P], BF)
                        nc.tensor.transpose(
                            pst[:], vT_raw[:, j * P:(j + 1) * P], ident[:])
                        for g in range(2):
                            nc.vector.tensor_copy(
                                v1[g][:, j, HD:P],
                                pst[:, g * HD:(g + 1) * HD])
                    return emit

                for jj in range(4):
                    chunks.append(mk_tr(4 * st + jj))
                add_q(2)
                add_q(3)
                return chunks

            # ---------------- wo machinery ----------------
            cct_tiles = {}

            def cct_load(t, u):
                """Load gathered unit u of tile t (half for t<3, pair
                for t=3) into sbuf.  wo_sb block index for gathered
                block (u, r): per-pair order 4m+r with m = 2hh+o2."""
                if t < NQT - 1:
                    cct = run.tile([P, 4, 2, QT], BF, tag="cct", bufs=2,
                                   name=f"cct{t}_{u}")
                    cc3 = cc_out[t][u][:].rearrange("(r o p) s -> p r o s",
                                                    p=P, o=2)
                else:
                    cct = run.tile([P, 4, QT], BF, tag="cct3", bufs=4,
                                   name=f"cct{t}_{u}")
                    cc3 = cc_out[t][u][:].rearrange("(r p) s -> p r s", p=P)
                nc.gpsimd.dma_start(cct[:], cc3[:])
                cct_tiles[(t, u)] = cct

            def wo_fin(t, d, pw):
                qsl = slice(t * QT, (t + 1) * QT)
                ot = run.tile([P, QT], F32, tag="ot", bufs=2, name="ot")
                nc.vector.tensor_copy(ot[:], pw[:])
                nc.sync.dma_start(out_t[d * P:(d + 1) * P, qsl], ot[:])

            def wo_chunks(t):
                """Filler chunks for the wo projection of q tile t<3,
                contracting the two gathered halves in sequence."""
                chunks = []
                state = {}

                def mk(d, hh, half, fin=False):
                    def emit():
                        if d not in state:
                            state[d] = wob(f"wo_{t}_d{d}")
                        pw = state[d]
                        cct = cct_tiles[(t, hh)]
                        for rr in range(2):
                            r = 2 * half + rr
                            for o2 in range(2):
                                nc.tensor.matmul(
                                    pw[:],
                                    wo_sb[:, 4 * (2 * hh + o2) + r,
                                          d * P:(d + 1) * P],
                                    cct[:, r, o2, :],
                                    start=(hh == 0 and r == 0 and o2 == 0),
                                    stop=(hh == 1 and r == 3 and o2 == 1))
                        if fin:
                            wo_fin(t, d, pw)
                    return emit

                early = [mk(0, 0, 0), mk(0, 0, 1), mk(1, 0, 0), mk(1, 0, 1)]
                late = [mk(0, 1, 0), mk(0, 1, 1, fin=True),
                        mk(1, 1, 0), mk(1, 1, 1, fin=True),
                        mk(2, 0, 0), mk(2, 0, 1), mk(3, 0, 0), mk(3, 0, 1),
                        mk(2, 1, 0), mk(2, 1, 1, fin=True),
                        mk(3, 1, 0), mk(3, 1, 1, fin=True)]
                return early, late

            def wo_tail_chunks(t):
                """wo for the last tile from the per-pair gathers."""
                chunks = []
                state = {}

                def mk(d, m, fin=False):
                    def emit():
                        if d not in state:
                            if d >= 2:
                                if "d23" not in state:
                                    state["d23"] = sc2(f"wo_{t}_d23")
                                state[d] = state["d23"][:, d - 2, :]
                            else:
                                state[d] = wob(f"wo_{t}_d{d}")
                        pw = state[d]
                        cct = cct_tiles[(t, m)]
                        for r in range(4):
                            nc.tensor.matmul(
                                pw[:],
                                wo_sb[:, 4 * m + r, d * P:(d + 1) * P],
                                cct[:, r, :],
                                start=(m == 0 and r == 0),
                                stop=(m == 3 and r == 3))
                        if fin:
                            wo_fin(t, d, pw)
                    return emit

                for m in range(3):
                    chunks += [mk(0, m), mk(1, m)]
                for m in range(3):
                    chunks += [mk(2, m), mk(3, m)]
                chunks += [mk(0, 3, fin=True), mk(1, 3, fin=True),
                           mk(2, 3, fin=True), mk(3, 3, fin=True)]
                return chunks

            # ---------------- attention ----------------
            def trig_ag(t, m):
                nc.gpsimd.collective_compute(
                    "AllGather",
                    mybir.AluOpType.bypass,
                    replica_groups=[[0, 1, 2, 3], [4, 5, 6, 7]],
                    ins=[cc_in[t][m][:].opt()],
                    outs=[cc_out[t][m][:].opt()],
                )

            prs = [slice(0, HD), slice(HD, P)]

            def attn_pair(t, m, pop):
                """Heads (m, m+4): hf=0 on partitions 0:64 vs kv group 0,
                hf=1 on 64:128 vs group 1 -- disjoint PE row groups, so
                each score matmul pair runs concurrently."""
                ngrp = 2 * (t + 1)
                qsl = slice(t * QT, (t + 1) * QT)
                pspv = [pvb(f"pv_{t}_{m}_{hf}") for hf in range(2)]
                e_pair = []
                for g2 in range(ngrp):
                    pss = [sc2(f"ss_{t}_{m}_{g2}_{hf}") for hf in range(2)]
                    for i in range(2):
                        j = 2 * g2 + i
                        for hf in range(2):
                            nc.tensor.matmul(
                                pss[hf][:, i, :],
                                k_fin[prs[hf], j * P:(j + 1) * P],
                                q_fin[m][prs[hf], qsl],
                                start=True, stop=True)
                    e2 = []
                    for hf in range(2):
                        e = run.tile([P, 2, QT], BF, tag="exp", bufs=6,
                                     name="e2")
                        nc.scalar.activation(e[:], pss[hf][:], Exp,
                                             scale=0.125)
                        cpair = g2 - 2 * t
                        if cpair >= 0:
                            nc.vector.tensor_mul(
                                e[:], e[:], msk[:, 2 * cpair:2 * cpair + 2, :])
                        e2.append(e)
                    e_pair.append(e2)
                    if g2 >= 1:
                        gp = g2 - 1
                        for i in range(2):
                            j = 2 * gp + i
                            for hf in range(2):
                                nc.tensor.matmul(
                                    pspv[hf][:], v1[hf][:, j, :],
                                    e_pair[gp][hf][:, i, :],
                                    start=(j == 0), stop=False)
                    pop()
                for i in range(2):
                    j = 2 * (ngrp - 1) + i
                    for hf in range(2):
                        nc.tensor.matmul(
                            pspv[hf][:], v1[hf][:, j, :],
                            e_pair[ngrp - 1][hf][:, i, :],
                            start=(j == 0), stop=(j == 4 * t + 3))
                # normalize: psum partitions 0:64 hold the row sums
                # (ones columns), 64:128 hold PV.
                for hf in range(2):
                    ocp = run.tile([P, QT], F32, tag="ocp", bufs=3,
                                   name="ocp")
                    nc.vector.tensor_copy(ocp[:], pspv[hf][:])
                    recip = run.tile([1, QT], F32, tag="recip", bufs=2,
                                     name="recip")
                    nc.vector.reciprocal_approx_fast(recip[:], ocp[0:1, :])
                    rb = dram.tile([1, QT], F32, tag="rb", bufs=2, name="rb")
                    nc.sync.dma_start(rb[:], recip[:])
                    bcast = run.tile([P, QT], F32, tag="bcast", bufs=2,
                                     name="bcast")
                    nc.sync.dma_start(bcast[HD:P, :],
                                      rb[:].to_broadcast((HD, QT)))
                    o_sb = run.tile([P, QT], BF, tag="osb", bufs=3,
                                    name="o_sb")
                    nc.vector.tensor_mul(o_sb[HD:P, :], ocp[HD:P, :],
                                         bcast[HD:P, :])
                    if t < NQT - 1:
                        row = (m % 2) * P + hf * HD
                        nc.sync.dma_start(
                            cc_in[t][m // 2][row:row + HD, :], o_sb[HD:P, :])
                    else:
                        nc.sync.dma_start(
                            cc_in[t][m][hf * HD:(hf + 1) * HD, :],
                            o_sb[HD:P, :])

            # ---------------- prologue: projections for s-tile 0 ------
            for emit in proj_chunks(0):
                emit()
            x_load(1)

            # ---------------- main pipelined loop ----------------
            for t in range(NQT):
                fillers = deque()
                wo_late = []
                if t < NQT - 1:
                    fillers.extend(proj_chunks(t + 1))
                if t >= 1:
                    early, wo_late = wo_chunks(t - 1)
                    fillers.extend(early)
                if t >= 1:
                    cct_load(t - 1, 0)
                if t + 2 < NQT:
                    x_load(t + 2)

                pops_total = 4 * 2 * (t + 1)
                pops_done = 0

                def pop():
                    nonlocal pops_done
                    pops_done += 1
                    left = pops_total - pops_done
                    if left <= 0:
                        while fillers:
                            fillers.popleft()()
                        return
                    k = -(-len(fillers) // (left + 1))
                    for _ in range(min(k, len(fillers))):
                        fillers.popleft()()

                # gpsimd queue: triggers interleave with the loads so a
                # slow AllGather's load-wait never delays a trigger
                # whose input is already on DRAM.
                last = t == NQT - 1
                for m in range(4):
                    attn_pair(t, m, pop)
                    if last:
                        trig_ag(t, m)
                        if m == 0 and t >= 1:
                            cct_load(t - 1, 1)
                            fillers.extend(wo_late)
                        if m == 2:
                            cct_load(t, 0)
                    else:
                        if m == 1:
                            trig_ag(t, 0)
                            if t >= 1:
                                cct_load(t - 1, 1)
                        if m == 2:
                            fillers.extend(wo_late)
                        if m == 3:
                            trig_ag(t, 1)
                while fillers:
                    fillers.popleft()()

            # ---------------- tail: wo for the last tile ----------------
            for m in range(1, 4):
                cct_load(NQT - 1, m)
            for emit in wo_tail_chunks(NQT - 1):
                emit()

    nc.compile()
    return nc


def _prep_inputs(x, position_ids, wq, wk, wv, wo):
    import ml_dtypes

    BF = ml_dtypes.bfloat16
    x = np.asarray(x, dtype=np.float32)
    pos = np.asarray(position_ids).reshape(-1).astype(np.int64)
    wqf = np.asarray(wq, dtype=np.float32)
    wkf = np.asarray(wk, dtype=np.float32)
    wvf = np.asarray(wv, dtype=np.float32)
    wof = np.asarray(wo, dtype=np.float32)

    inv = 1.0 / (ROPE_BASE ** (np.arange(0, HD, 2, dtype=np.float32) / HD))
    freqs = np.outer(pos.astype(np.float32), inv)  # [S, 32]
    pidx = np.arange(P) % 32
    sign = np.where((np.arange(P) % HD) < 32, -1.0, 1.0).astype(np.float32)
    cosT = np.cos(freqs)[:, pidx].T                                # [P, S]
    sinT = np.sin(freqs)[:, pidx].T * sign[:, None]
    csT = np.ascontiguousarray(
        np.stack([cosT, sinT], axis=1).astype(np.float32))  # [P, 2, S]

    pg = np.arange(P)[:, None, None]
    cg = np.arange(4)[None, :, None]
    fg = np.arange(QT)[None, None, :]
    maskT = ((fg - pg - 128 * cg) >= 0).astype(BF)

    def tile_po(wT, fw):
        """[dim, f] -> [p, o, f] with dim = o*128 + p, contiguous."""
        return np.ascontiguousarray(
            wT.reshape(DK, P, fw).transpose(1, 0, 2).astype(BF))

    # x pre-tiled to [st, p, o, s]
    xH = [np.ascontiguousarray(
        x[b].reshape(NQT, QT, DK, P).transpose(0, 3, 2, 1).astype(BF))
        for b in range(B)]

    in_maps = []
    for c in range(N_CORES):
        b, k = c // 4, c % 4
        # wq columns in head-paired order: block m = [head 8k+m | 8k+m+4]
        wq_t = np.empty((4, P, DK, P), dtype=BF)
        for m in range(4):
            qcols = []
            for hf in range(2):
                h = 8 * k + m + 4 * hf
                qcols.extend(range(h * HD, (h + 1) * HD))
            wq_t[m] = tile_po(wqf[qcols, :].T, P)
        # combined [p, 4*DK, 128]: block m at columns m*DK..(m+1)*DK
        wqT_loc = np.ascontiguousarray(
            wq_t.transpose(1, 0, 2, 3).reshape(P, 4 * DK, P))
        wkT_loc = tile_po(wkf[2 * k * HD:(2 * k + 2) * HD, :].T, P)
        wvT_loc = tile_po(wvf[2 * k * HD:(2 * k + 2) * HD, :].T, P)
        wkvT_loc = np.ascontiguousarray(
            np.concatenate([wkT_loc, wvT_loc], axis=1))
        # wo rows permuted to the per-pair gathered layout: block
        # o = 4m + r covers rows of head 8r + 4hf + m at hf*64+d.
        perm = []
        for m in range(4):
            for r in range(4):
                for hf in range(2):
                    h = 8 * r + 4 * hf + m
                    perm.extend(range(h * HD, (h + 1) * HD))
        woT_loc = tile_po(wof[512 * k:512 * (k + 1), perm].T, 512)
        in_maps.append({
            "xT": xH[b],
            "wqT": wqT_loc,
            "wkvT": wkvT_loc,
            "woT": woT_loc,
            "csT": csT,
            "maskT": maskT,
        })
    return in_maps


LAST_EXEC_NS = None


def kernel(x, position_ids, wq, wk, wv, wo, _trace=False):
    import time

    from concourse import bass_utils

    if "nc" not in _CACHE:
        _CACHE["nc"] = _build()
    nc = _CACHE["nc"]

    in_maps = _prep_inputs(x, position_ids, wq, wk, wv, wo)
    res = None
    for attempt in range(3):
        try:
            res = bass_utils.run_bass_kernel_spmd(
                nc, in_maps, core_ids=list(range(N_CORES)), trace=_trace)
            break
        except Exception:
            if attempt == 2:
                raise
            time.sleep(20 * (attempt + 1))

    global LAST_EXEC_NS
    LAST_EXEC_NS = res.exec_time_ns

    out = np.empty((B, S, DIM), dtype=np.float32)
    for c in range(N_CORES):
        b, k = c // 4, c % 4
        out[b, :, 512 * k:512 * (k + 1)] = res.results[c]["out_t"].T
    return out
